# revision 7
# baseline (speedup 1.0000x reference)
"""Trainium2 Bass kernel for nn_Absolute_attention (sparse_attention).

Reference math (b=4, l=4096, dim=1024, h=16, hd=64):
    q = softmax((x @ Wq.T).reshape(b,l,h,hd+1), -1)
    time encoding: qk_weight = (1-q[...,-1]) * sum_d(time^2)  where
        sum_d(time[l,h,:]^2) = inv_hd * sum_j((c+s)^2 + (c-s)^2) = 2 exactly,
        so qk_weight = 2*(1-q_last)  (time/cos/sin cancel analytically).
    k = softmax((x @ Wk.T).reshape(b,l,h,hd), -1) * mask
    v = x @ Wv.T
    out = ((qk_weight[...,None]*k).reshape(b,l,h*hd) * v) @ Wo.T + bo

Everything is pointwise per (b,l) row -> pure data-parallel row sharding:
16384 rows over 8 cores = 2048 rows/core, 16 blocks of 128 rows.

Per 128-row block (layout: rows on partitions):
    z = x_blk @ Wcat.T  (Wcat = [Wq;Wk;Wv], 3088 cols) via PE fp32r matmuls,
        contraction over dim in 8 chunks of 128 (stationary = x.T chunks).
    e = exp(z[:, :2064])  (q+k logits; softmax without max-subtraction --
        logits are O(+-4), exp is safe in fp32)
    denq = segmented sum e_q (16 groups of 65); denk = seg sum e_k (16x64)
    G = 2*mask*(denq - eq_last) / (denq*denk)
    a = e_k * v * G[head-broadcast]   (fp32r)
    aT = PE transpose of a (8x 128x128)
    out = aT.T @ Wo.T + bo  via PE fp32r, then DMA out.

All matmul operands are fp16 (11-bit mantissa, ~3e-4 matmul err; 2-byte
weight loads keep the PE stream-bound and halve the weight DMA).
The first two blocks are processed chunk-major so compute overlaps the
16.8MB weight DMA stream instead of stalling behind it.
"""
import numpy as np

import concourse.bacc as bacc
import concourse.mybir as mybir
import concourse.tile as tile
from concourse.bass_utils import run_bass_kernel_spmd

FP32 = mybir.dt.float32
F32R = mybir.dt.float32r
F16 = mybir.dt.float16
AX = mybir.AxisListType.X
ADD = mybir.AluOpType.add
EXP = mybir.ActivationFunctionType.Exp

B, L, DIM, H, HD = 4, 4096, 1024, 16, 64
ROWS = B * L                      # 16384
NCORES = 8
CROWS = ROWS // NCORES            # 2048
NBLK = CROWS // 128               # 16
NQ = H * (HD + 1)                 # 1040
NK = H * HD                       # 1024
QK = NQ + NK                      # 2064
TOT = QK + NK                     # 3088 (q | k | v)
NDC = DIM // 128                  # 8 contraction chunks

# N-chunks of the projection output; each fits one PSUM bank.
# First 5 cover the exp region [0, 2064), last 2 cover v.
CHUNKS = [(0, 512), (512, 512), (1024, 512), (1536, 512), (2048, 16),
          (2064, 512), (2576, 512)]
WTBASE = []
_acc = 0
for _off, _sz in CHUNKS:
    WTBASE.append(_acc)
    _acc += NDC * _sz
WTCOLS = _acc                     # 24704

WARM = 3                          # blocks processed chunk-major at start

_CACHE = {}


def _build():
    nc = bacc.Bacc("TRN2", target_bir_lowering=False, debug=False)
    xt_d = nc.dram_tensor("xt", [NBLK, 128, 1024], F16, kind="ExternalInput").ap()
    wt_d = nc.dram_tensor("wt", [128, WTCOLS], F16, kind="ExternalInput").ap()
    wo_d = nc.dram_tensor("wo", [128, NDC * 1024], F16, kind="ExternalInput").ap()
    bo_d = nc.dram_tensor("bo", [128, 1024], FP32, kind="ExternalInput").ap()
    m_d = nc.dram_tensor("msk", [128, NBLK], FP32, kind="ExternalInput").ap()
    id_d = nc.dram_tensor("ident", [128, 128], F16, kind="ExternalInput").ap()
    out_d = nc.dram_tensor("out", [NBLK, 128, 1024], FP32, kind="ExternalOutput").ap()

    with tile.TileContext(nc) as tc:
        with (
            tc.tile_pool(name="const", bufs=1) as cp,
            tc.tile_pool(name="xp", bufs=5) as xp,
            tc.tile_pool(name="ep", bufs=3) as ep,
            tc.tile_pool(name="t1p", bufs=2) as t1p,
            tc.tile_pool(name="ap_", bufs=3) as apool,
            tc.tile_pool(name="atp", bufs=2) as atp,
            tc.tile_pool(name="op", bufs=2) as op,
            tc.tile_pool(name="sp", bufs=2) as sp,
            tc.tile_pool(name="pp", bufs=5, space="PSUM") as pp,
            tc.tile_pool(name="tp", bufs=1, space="PSUM") as tp,
            tc.tile_pool(name="outp", bufs=2, space="PSUM") as outp,
        ):
            wt = cp.tile([128, WTCOLS], F16, tag="wt")
            wo = cp.tile([128, NDC * 1024], F16, tag="wo")
            bo = cp.tile([128, 1024], FP32, tag="bo")
            msk = cp.tile([128, NBLK], FP32, tag="msk")
            ident = cp.tile([128, 128], F16, tag="ident")

            def load_wt(k):
                lo, hi = WTBASE[k], WTBASE[k] + NDC * CHUNKS[k][1]
                nc.sync.dma_start(wt[:, lo:hi], wt_d[:, lo:hi])

            def load_xt(i):
                t = xp.tile([128, 1024], F16, tag="xt")
                nc.sync.dma_start(t[:], xt_d[i])
                return t

            # DMA issue order tuned so data arrives roughly when needed:
            # x for the warmup blocks + early weight chunks first.
            xts = {0: load_xt(0)}
            load_wt(0)
            load_wt(1)
            xts[1] = load_xt(1)
            load_wt(2)
            load_wt(3)
            nc.sync.dma_start(msk[:], m_d[:])
            nc.sync.dma_start(ident[:], id_d[:])
            load_wt(4)
            load_wt(5)
            xts[2] = load_xt(2)
            load_wt(6)
            xts[3] = load_xt(3)
            nc.sync.dma_start(wo[:], wo_d[:])
            nc.sync.dma_start(bo[:], bo_d[:])

            def proj_chunk(xt, k):
                """Accumulate projection chunk k into a psum tile."""
                off, sz = CHUNKS[k]
                ps = pp.tile([128, sz], FP32, tag="pp")
                for c in range(NDC):
                    nc.tensor.matmul(
                        ps[:], xt[:, c * 128:(c + 1) * 128],
                        wt[:, WTBASE[k] + c * sz: WTBASE[k] + (c + 1) * sz],
                        start=(c == 0), stop=(c == NDC - 1))
                return ps

            def proj_pair45(xt):
                """Chunks 4 (16-wide) and 5 interleaved so the tiny chunk's
                weight loads hide under 512-wide streams."""
                off4, sz4 = CHUNKS[4]
                ps4 = pp.tile([128, sz4], FP32, tag="pp")
                ps5 = pp.tile([128, CHUNKS[5][1]], FP32, tag="pp")
                for c in range(NDC):
                    nc.tensor.matmul(
                        ps5[:], xt[:, c * 128:(c + 1) * 128],
                        wt[:, WTBASE[5] + c * 512: WTBASE[5] + (c + 1) * 512],
                        start=(c == 0), stop=(c == NDC - 1))
                    nc.tensor.matmul(
                        ps4[:], xt[:, c * 128:(c + 1) * 128],
                        wt[:, WTBASE[4] + c * sz4: WTBASE[4] + (c + 1) * sz4],
                        start=(c == 0), stop=(c == NDC - 1))
                return ps4, ps5

            def finish_block(i, xt, e, ps5):
                """Everything after the exp chunks: v, stats, gate, a."""
                ps6 = proj_chunk(xt, 6)

                # free the v psums first: t1 = e_k * v
                t1 = t1p.tile([128, 1024], FP32, tag="t1")
                nc.vector.tensor_mul(t1[:, 0:512], e[:, QK - 1024:QK - 512], ps5[:])
                nc.vector.tensor_mul(t1[:, 512:1024], e[:, QK - 512:QK], ps6[:])

                eq = e[:, 0:NQ].rearrange("p (h j) -> p h j", j=HD + 1)
                ek = e[:, NQ:QK].rearrange("p (h j) -> p h j", j=HD)
                denq = sp.tile([128, H], FP32, tag="denq")
                denk = sp.tile([128, H], FP32, tag="denk")
                eql = sp.tile([128, H], FP32, tag="eql")
                g = sp.tile([128, H], FP32, tag="g")
                nc.vector.tensor_reduce(denq[:], eq, axis=AX, op=ADD)
                nc.vector.tensor_reduce(denk[:], ek, axis=AX, op=ADD)
                nc.vector.tensor_copy(eql[:], eq[:, :, HD])
                nc.vector.tensor_sub(g[:], denq[:], eql[:])        # denq-eqlast
                nc.vector.tensor_mul(denq[:], denq[:], denk[:])    # denq*denk
                nc.vector.reciprocal(denk[:], denq[:])             # 1/(dq*dk)
                nc.vector.tensor_mul(g[:], g[:], denk[:])
                # msk holds 2*attention_mask -> G = 2*mask*(dq-el)/(dq*dk)
                nc.vector.tensor_scalar_mul(g[:], g[:], msk[:, i:i + 1])

                a = apool.tile([128, 1024], F16, tag="a")
                nc.vector.tensor_mul(
                    a[:].rearrange("p (h j) -> p h j", j=HD),
                    t1[:].rearrange("p (h j) -> p h j", j=HD),
                    g[:].to_broadcast((128, H, HD)))
                return a

            def tail(st):
                """Transpose a -> aT, final matmul + bias, DMA out."""
                i, a = st
                at = atp.tile([128, 1024], F16, tag="at")
                tps = tp.tile([128, 1024], F16, tag="tp")
                for c in range(NDC):
                    nc.tensor.matmul(tps[:, c * 128:(c + 1) * 128],
                                     a[:, c * 128:(c + 1) * 128], ident[:],
                                     is_transpose=True, skip_group_check=True)
                nc.scalar.copy(at[:, 0:512], tps[:, 0:512])
                nc.scalar.copy(at[:, 512:1024], tps[:, 512:1024])
                outsb = op.tile([128, 1024], FP32, tag="outsb")
                for half in range(2):
                    ops = outp.tile([128, 512], FP32, tag="outp")
                    for c in range(NDC):
                        nc.tensor.matmul(
                            ops[:], at[:, c * 128:(c + 1) * 128],
                            wo[:, c * 1024 + half * 512: c * 1024 + half * 512 + 512],
                            start=(c == 0), stop=(c == NDC - 1))
                    nc.vector.tensor_add(outsb[:, half * 512:(half + 1) * 512],
                                         ops[:], bo[:, half * 512:(half + 1) * 512])
                nc.sync.dma_start(out_d[i], outsb[:])

            # ---- warmup: blocks 0..WARM-1 chunk-major, tracking the
            # weight-chunk DMA arrival order ----
            es = {i: ep.tile([128, QK], FP32, tag="e", name="e") for i in range(WARM)}
            for k in range(4):
                for i in range(WARM):
                    ps = proj_chunk(xts[i], k)
                    off, sz = CHUNKS[k]
                    nc.scalar.activation(es[i][:, off:off + sz], ps[:], EXP)
            ps5s = {}
            for i in range(WARM):
                ps4, ps5s[i] = proj_pair45(xts[i])
                nc.scalar.activation(es[i][:, 2048:2064], ps4[:], EXP)
            pending = []
            for i in range(WARM):
                pending.append((i, finish_block(i, xts[i], es[i], ps5s[i])))

            # ---- steady state ----
            for i in range(WARM, NBLK):
                xt = xts.get(i) or load_xt(i)
                e = ep.tile([128, QK], FP32, tag="e")
                for k in range(4):
                    off, sz = CHUNKS[k]
                    ps = proj_chunk(xt, k)
                    nc.scalar.activation(e[:, off:off + sz], ps[:], EXP)
                ps4, ps5 = proj_pair45(xt)
                nc.scalar.activation(e[:, 2048:2064], ps4[:], EXP)
                pending.append((i, finish_block(i, xt, e, ps5)))
                tail(pending.pop(0))
            while pending:
                tail(pending.pop(0))
    nc.compile()
    return nc


def _host_prep(x, attention_mask, Wq, Wk, Wv, Wo, bo):
    x_flat = np.ascontiguousarray(np.asarray(x, dtype=np.float32)).reshape(ROWS, DIM)
    Wcat_T = np.ascontiguousarray(
        np.concatenate([np.asarray(Wq, np.float32), np.asarray(Wk, np.float32),
                        np.asarray(Wv, np.float32)], axis=0).T)  # [1024, 3088]
    cols = []
    for off, sz in CHUNKS:
        for c in range(NDC):
            cols.append(Wcat_T[c * 128:(c + 1) * 128, off:off + sz])
    wt_host = np.ascontiguousarray(np.concatenate(cols, axis=1)).astype(np.float16)

    wo_host = np.ascontiguousarray(
        np.asarray(Wo, np.float32).T.reshape(NDC, 128, 1024)
        .transpose(1, 0, 2).reshape(128, NDC * 1024)).astype(np.float16)
    bo_host = np.ascontiguousarray(
        np.broadcast_to(np.asarray(bo, np.float32), (128, 1024)))
    id_host = np.eye(128, dtype=np.float16)
    m_flat = (2.0 * np.asarray(attention_mask, np.float32)).reshape(ROWS)

    in_maps = []
    for i in range(NCORES):
        sl = slice(i * CROWS, (i + 1) * CROWS)
        xt = np.ascontiguousarray(
            x_flat[sl].reshape(NBLK, 128, NDC, 128).transpose(0, 3, 2, 1)
        ).reshape(NBLK, 128, 1024).astype(np.float16)
        mc = np.ascontiguousarray(m_flat[sl].reshape(NBLK, 128).T)
        in_maps.append({"xt": xt, "wt": wt_host, "wo": wo_host,
                        "bo": bo_host, "msk": mc, "ident": id_host})
    return in_maps


def run(inputs, trace=False):
    """Run the kernel; returns (output, exec_time_ns or None)."""
    if "nc" not in _CACHE:
        _CACHE["nc"] = _build()
    nc = _CACHE["nc"]
    in_maps = _host_prep(
        inputs["x"], inputs["attention_mask"], inputs["Wq"], inputs["Wk"],
        inputs["Wv"], inputs["Wo"], inputs["bo"])
    res = run_bass_kernel_spmd(nc, in_maps, list(range(NCORES)), trace=trace)
    out = np.concatenate(
        [res.results[i]["out"].reshape(CROWS, DIM) for i in range(NCORES)],
        axis=0).reshape(B, L, DIM)
    return out, res.exec_time_ns


def kernel(**inputs) -> np.ndarray:
    assert inputs["x"].shape == (B, L, DIM)
    out, _ = run(inputs, trace=False)
    return out


# revision 17
# speedup vs baseline: 1.0430x; 1.0430x over previous
"""Trainium2 Bass kernel for nn_Absolute_attention (sparse_attention).

Reference math (b=4, l=4096, dim=1024, h=16, hd=64):
    q = softmax((x @ Wq.T).reshape(b,l,h,hd+1), -1)
    time encoding: qk_weight = (1-q[...,-1]) * sum_d(time^2)  where
        sum_d(time[l,h,:]^2) = inv_hd * sum_j((c+s)^2 + (c-s)^2) = 2 exactly,
        so qk_weight = 2*(1-q_last)  (time/cos/sin cancel analytically).
    k = softmax((x @ Wk.T).reshape(b,l,h,hd), -1) * mask
    v = x @ Wv.T
    out = ((qk_weight[...,None]*k).reshape(b,l,h*hd) * v) @ Wo.T + bo

Everything is pointwise per (b,l) row -> pure data-parallel row sharding:
16384 rows over 8 cores = 2048 rows/core, 16 blocks of 128 rows.

Per 128-row block (layout: rows on partitions):
    z = x_blk @ Wcat.T  (Wcat = [Wq;Wk;Wv], 3088 cols) via PE fp16 matmuls,
        contraction over dim in 8 chunks of 128 (stationary = x.T chunks).
    e = exp(z[:, :2064])  (q+k logits; softmax without max-subtraction --
        logits are O(+-4), exp is safe in fp32)
    denq = segmented sum e_q (16 groups of 65); denk = seg sum e_k (16x64)
    G = 2*mask*(denq - eq_last) / (denq*denk)
    a = e_k * v * G[head-broadcast]   (fp16)
    aT = PE transpose of a (8x 128x128, into one fp16 PSUM bank)
    out = aT.T @ Wo.T + bo  via PE fp16 matmuls, then DMA out.

All matmul operands are fp16 (11-bit mantissa, ~3e-4 matmul err; 2-byte
weight loads keep the PE stream-bound and halve the weight DMA).
The first two blocks are processed chunk-major so compute overlaps the
8.4MB weight DMA stream instead of stalling behind it; later blocks run a
software pipeline (transposes of block i-2 -> projection of block i ->
final matmul of block i-2) that keeps the PE stream-bound.
"""
import numpy as np

import concourse.bacc as bacc
import concourse.mybir as mybir
import concourse.tile as tile
from concourse.bass_utils import run_bass_kernel_spmd

FP32 = mybir.dt.float32
F32R = mybir.dt.float32r
F16 = mybir.dt.float16
AX = mybir.AxisListType.X
ADD = mybir.AluOpType.add
EXP = mybir.ActivationFunctionType.Exp

B, L, DIM, H, HD = 4, 4096, 1024, 16, 64
ROWS = B * L                      # 16384
NCORES = 8
CROWS = ROWS // NCORES            # 2048
NBLK = CROWS // 128               # 16
NQ = H * (HD + 1)                 # 1040
NK = H * HD                       # 1024
QK = NQ + NK                      # 2064
TOT = QK + NK                     # 3088 (q | k | v)
NDC = DIM // 128                  # 8 contraction chunks

# N-chunks of the projection output; each fits one PSUM bank.
# First 5 cover the exp region [0, 2064), last 2 cover v. All exp chunks
# are >=256 wide so fp16 LDWEIGHTS (~100ns) hides fully under the streams.
CHUNKS = [(0, 416), (416, 416), (832, 416), (1248, 416), (1664, 400),
          (2064, 512), (2576, 512)]
WTBASE = []
_acc = 0
for _off, _sz in CHUNKS:
    WTBASE.append(_acc)
    _acc += NDC * _sz
WTCOLS = _acc                     # 24704

WARM = 2                          # blocks processed chunk-major at start

_CACHE = {}


def _build():
    nc = bacc.Bacc("TRN2", target_bir_lowering=False, debug=False)
    xt_d = nc.dram_tensor("xt", [NBLK, 128, 1024], F16, kind="ExternalInput").ap()
    wt_d = nc.dram_tensor("wt", [128, WTCOLS], F16, kind="ExternalInput").ap()
    wo_d = nc.dram_tensor("wo", [128, NDC * 1024], F16, kind="ExternalInput").ap()
    bo_d = nc.dram_tensor("bo", [128, 1024], FP32, kind="ExternalInput").ap()
    m_d = nc.dram_tensor("msk", [128, NBLK], FP32, kind="ExternalInput").ap()
    id_d = nc.dram_tensor("ident", [128, 128], F16, kind="ExternalInput").ap()
    out_d = nc.dram_tensor("out", [NBLK, 128, 1024], FP32, kind="ExternalOutput").ap()

    with tile.TileContext(nc) as tc:
        with (
            tc.tile_pool(name="const", bufs=1) as cp,
            tc.tile_pool(name="xp", bufs=3) as xp,
            tc.tile_pool(name="ep", bufs=3) as ep,
            tc.tile_pool(name="t1p", bufs=2) as t1p,
            tc.tile_pool(name="ap_", bufs=3) as apool,
            tc.tile_pool(name="atp", bufs=2) as atp,
            tc.tile_pool(name="op", bufs=2) as op,
            tc.tile_pool(name="sp", bufs=2) as sp,
            tc.tile_pool(name="pp", bufs=5, space="PSUM") as pp,
            tc.tile_pool(name="tp", bufs=1, space="PSUM") as tp,
            tc.tile_pool(name="outp", bufs=2, space="PSUM") as outp,
        ):
            wt = cp.tile([128, WTCOLS], F16, tag="wt")
            wo = cp.tile([128, NDC * 1024], F16, tag="wo")
            bo = cp.tile([128, 1024], FP32, tag="bo")
            msk = cp.tile([128, NBLK], FP32, tag="msk")
            ident = cp.tile([128, 128], F16, tag="ident")

            def load_wt(k):
                lo, hi = WTBASE[k], WTBASE[k] + NDC * CHUNKS[k][1]
                nc.sync.dma_start(wt[:, lo:hi], wt_d[:, lo:hi])

            def load_xt(i):
                t = xp.tile([128, 1024], F16, tag="xt")
                nc.sync.dma_start(t[:], xt_d[i])
                return t

            # DMA issue order tuned so data arrives roughly when needed:
            # x for the warmup blocks + early weight chunks first.
            xts = {0: load_xt(0)}
            load_wt(0)
            load_wt(1)
            xts[1] = load_xt(1)
            load_wt(2)
            load_wt(3)
            nc.sync.dma_start(msk[:], m_d[:])
            nc.sync.dma_start(ident[:], id_d[:])
            load_wt(4)
            load_wt(5)
            xts[2] = load_xt(2)
            load_wt(6)
            nc.sync.dma_start(wo[:], wo_d[:])
            nc.sync.dma_start(bo[:], bo_d[:])

            def proj_chunk(xt, k):
                """Accumulate projection chunk k into a psum tile."""
                off, sz = CHUNKS[k]
                ps = pp.tile([128, sz], FP32, tag="pp")
                for c in range(NDC):
                    nc.tensor.matmul(
                        ps[:], xt[:, c * 128:(c + 1) * 128],
                        wt[:, WTBASE[k] + c * sz: WTBASE[k] + (c + 1) * sz],
                        start=(c == 0), stop=(c == NDC - 1))
                return ps

            def finish_block(i, xt, e):
                """Everything after the exp chunks: v, stats, gate, a."""
                ps5 = proj_chunk(xt, 5)
                ps6 = proj_chunk(xt, 6)

                eq = e[:, 0:NQ].rearrange("p (h j) -> p h j", j=HD + 1)
                ek = e[:, NQ:QK].rearrange("p (h j) -> p h j", j=HD)
                denq = sp.tile([128, H], FP32, tag="denq")
                denk = sp.tile([128, H], FP32, tag="denk")
                eql = sp.tile([128, H], FP32, tag="eql")
                g = sp.tile([128, H], FP32, tag="g")
                nc.vector.tensor_reduce(denq[:], eq, axis=AX, op=ADD)
                nc.vector.tensor_reduce(denk[:], ek, axis=AX, op=ADD)
                nc.vector.tensor_copy(eql[:], eq[:, :, HD])
                nc.vector.tensor_sub(g[:], denq[:], eql[:])        # denq-eqlast
                nc.vector.tensor_mul(denq[:], denq[:], denk[:])    # denq*denk
                nc.vector.reciprocal(denk[:], denq[:])             # 1/(dq*dk)
                # msk holds 2*attention_mask -> G = 2*mask*(dq-el)/(dq*dk)
                nc.vector.scalar_tensor_tensor(
                    g[:], g[:], msk[:, i:i + 1], denk[:],
                    op0=mybir.AluOpType.mult, op1=mybir.AluOpType.mult)

                t1 = t1p.tile([128, 1024], FP32, tag="t1")
                nc.vector.tensor_mul(t1[:, 0:512], e[:, QK - 1024:QK - 512], ps5[:])
                nc.vector.tensor_mul(t1[:, 512:1024], e[:, QK - 512:QK], ps6[:])
                a = apool.tile([128, 1024], F16, tag="a")
                nc.vector.tensor_mul(
                    a[:].rearrange("p (h j) -> p h j", j=HD),
                    t1[:].rearrange("p (h j) -> p h j", j=HD),
                    g[:].to_broadcast((128, H, HD)))
                return a

            def tail(st, last=False):
                """Transpose a -> aT, final matmul + bias, DMA out."""
                i, a = st
                at = atp.tile([128, 1024], F16, tag="at")
                tps = tp.tile([128, 1024], F16, tag="tp")
                for c in range(NDC):
                    nc.tensor.matmul(tps[:, c * 128:(c + 1) * 128],
                                     a[:, c * 128:(c + 1) * 128], ident[:],
                                     is_transpose=True, skip_group_check=True)
                cp_eng = nc.vector.tensor_copy if last else nc.scalar.copy
                cp_eng(at[:, 0:512], tps[:, 0:512])
                cp_eng(at[:, 512:1024], tps[:, 512:1024])
                outsb = op.tile([128, 1024], FP32, tag="outsb")
                for half in range(2):
                    ops = outp.tile([128, 512], FP32, tag="outp")
                    for c in range(NDC):
                        nc.tensor.matmul(
                            ops[:], at[:, c * 128:(c + 1) * 128],
                            wo[:, c * 1024 + half * 512: c * 1024 + half * 512 + 512],
                            start=(c == 0), stop=(c == NDC - 1))
                    nc.vector.tensor_add(outsb[:, half * 512:(half + 1) * 512],
                                         ops[:], bo[:, half * 512:(half + 1) * 512])
                nc.sync.dma_start(out_d[i], outsb[:])

            # ---- warmup: blocks 0..WARM-1 chunk-major, tracking the
            # weight-chunk DMA arrival order ----
            es = {i: ep.tile([128, QK], F16, tag="e", name="e") for i in range(WARM)}
            for k in range(5):
                for i in range(WARM):
                    ps = proj_chunk(xts[i], k)
                    off, sz = CHUNKS[k]
                    nc.scalar.activation(es[i][:, off:off + sz], ps[:], EXP)
            pending = []
            for i in range(WARM):
                pending.append((i, finish_block(i, xts[i], es[i])))

            # ---- steady state ----
            for i in range(WARM, NBLK):
                xt = xts.get(i) or load_xt(i)
                e = ep.tile([128, QK], F16, tag="e")
                for k in range(4):
                    off, sz = CHUNKS[k]
                    ps = proj_chunk(xt, k)
                    nc.scalar.activation(e[:, off:off + sz], ps[:], EXP)
                ps4, ps5 = proj_pair45(xt)
                nc.scalar.activation(e[:, 2048:2064], ps4[:], EXP)
                pending.append((i, finish_block(i, xt, e, ps5)))
                tail(pending.pop(0))
            while pending:
                tail(pending.pop(0), last=True)
    nc.compile()
    return nc


def _host_prep(x, attention_mask, Wq, Wk, Wv, Wo, bo):
    x_flat = np.ascontiguousarray(np.asarray(x, dtype=np.float32)).reshape(ROWS, DIM)
    Wcat_T = np.ascontiguousarray(
        np.concatenate([np.asarray(Wq, np.float32), np.asarray(Wk, np.float32),
                        np.asarray(Wv, np.float32)], axis=0).T)  # [1024, 3088]
    cols = []
    for off, sz in CHUNKS:
        for c in range(NDC):
            cols.append(Wcat_T[c * 128:(c + 1) * 128, off:off + sz])
    wt_host = np.ascontiguousarray(np.concatenate(cols, axis=1)).astype(np.float16)

    wo_host = np.ascontiguousarray(
        np.asarray(Wo, np.float32).T.reshape(NDC, 128, 1024)
        .transpose(1, 0, 2).reshape(128, NDC * 1024)).astype(np.float16)
    bo_host = np.ascontiguousarray(
        np.broadcast_to(np.asarray(bo, np.float32), (128, 1024)))
    id_host = np.eye(128, dtype=np.float16)
    m_flat = (2.0 * np.asarray(attention_mask, np.float32)).reshape(ROWS)

    in_maps = []
    for i in range(NCORES):
        sl = slice(i * CROWS, (i + 1) * CROWS)
        xt = np.ascontiguousarray(
            x_flat[sl].reshape(NBLK, 128, NDC, 128).transpose(0, 3, 2, 1)
        ).reshape(NBLK, 128, 1024).astype(np.float16)
        mc = np.ascontiguousarray(m_flat[sl].reshape(NBLK, 128).T)
        in_maps.append({"xt": xt, "wt": wt_host, "wo": wo_host,
                        "bo": bo_host, "msk": mc, "ident": id_host})
    return in_maps


def run(inputs, trace=False):
    """Run the kernel; returns (output, exec_time_ns or None)."""
    if "nc" not in _CACHE:
        _CACHE["nc"] = _build()
    nc = _CACHE["nc"]
    in_maps = _host_prep(
        inputs["x"], inputs["attention_mask"], inputs["Wq"], inputs["Wk"],
        inputs["Wv"], inputs["Wo"], inputs["bo"])
    res = run_bass_kernel_spmd(nc, in_maps, list(range(NCORES)), trace=trace)
    out = np.concatenate(
        [res.results[i]["out"].reshape(CROWS, DIM) for i in range(NCORES)],
        axis=0).reshape(B, L, DIM)
    return out, res.exec_time_ns


def kernel(**inputs) -> np.ndarray:
    assert inputs["x"].shape == (B, L, DIM)
    out, _ = run(inputs, trace=False)
    return out


# revision 19
# speedup vs baseline: 1.0494x; 1.0062x over previous
"""Trainium2 Bass kernel for nn_Absolute_attention (sparse_attention).

Reference math (b=4, l=4096, dim=1024, h=16, hd=64):
    q = softmax((x @ Wq.T).reshape(b,l,h,hd+1), -1)
    time encoding: qk_weight = (1-q[...,-1]) * sum_d(time^2)  where
        sum_d(time[l,h,:]^2) = inv_hd * sum_j((c+s)^2 + (c-s)^2) = 2 exactly,
        so qk_weight = 2*(1-q_last)  (time/cos/sin cancel analytically).
    k = softmax((x @ Wk.T).reshape(b,l,h,hd), -1) * mask
    v = x @ Wv.T
    out = ((qk_weight[...,None]*k).reshape(b,l,h*hd) * v) @ Wo.T + bo

Everything is pointwise per (b,l) row -> pure data-parallel row sharding:
16384 rows over 8 cores = 2048 rows/core, 16 blocks of 128 rows.

Per 128-row block (layout: rows on partitions):
    z = x_blk @ Wcat.T  (Wcat = [Wq;Wk;Wv], 3088 cols) via PE fp32r matmuls,
        contraction over dim in 8 chunks of 128 (stationary = x.T chunks).
    e = exp(z[:, :2064])  (q+k logits; softmax without max-subtraction --
        logits are O(+-4), exp is safe in fp32)
    denq = segmented sum e_q (16 groups of 65); denk = seg sum e_k (16x64)
    G = 2*mask*(denq - eq_last) / (denq*denk)
    a = e_k * v * G[head-broadcast]   (fp32r)
    aT = PE transpose of a (8x 128x128)
    out = aT.T @ Wo.T + bo  via PE fp32r, then DMA out.

All matmul operands are fp16 (11-bit mantissa, ~3e-4 matmul err; 2-byte
weight loads keep the PE stream-bound and halve the weight DMA).
The first two blocks are processed chunk-major so compute overlaps the
16.8MB weight DMA stream instead of stalling behind it.
"""
import numpy as np

import concourse.bacc as bacc
import concourse.mybir as mybir
import concourse.tile as tile
from concourse.bass_utils import run_bass_kernel_spmd

FP32 = mybir.dt.float32
F32R = mybir.dt.float32r
F16 = mybir.dt.float16
AX = mybir.AxisListType.X
ADD = mybir.AluOpType.add
EXP = mybir.ActivationFunctionType.Exp

B, L, DIM, H, HD = 4, 4096, 1024, 16, 64
ROWS = B * L                      # 16384
NCORES = 8
CROWS = ROWS // NCORES            # 2048
NBLK = CROWS // 128               # 16
NQ = H * (HD + 1)                 # 1040
NK = H * HD                       # 1024
QK = NQ + NK                      # 2064
TOT = QK + NK                     # 3088 (q | k | v)
NDC = DIM // 128                  # 8 contraction chunks

# N-chunks of the projection output; each fits one PSUM bank.
# First 5 cover the exp region [0, 2064), last 2 cover v. All exp chunks
# are >=256 wide so fp16 LDWEIGHTS (~100ns) hides fully under the streams.
CHUNKS = [(0, 416), (416, 416), (832, 416), (1248, 416), (1664, 400),
          (2064, 512), (2576, 512)]
WTBASE = []
_acc = 0
for _off, _sz in CHUNKS:
    WTBASE.append(_acc)
    _acc += NDC * _sz
WTCOLS = _acc                     # 24704

WARM = 2                          # blocks processed chunk-major at start

_CACHE = {}


def _build():
    nc = bacc.Bacc("TRN2", target_bir_lowering=False, debug=False)
    xt_d = nc.dram_tensor("xt", [NBLK, 128, 1024], F16, kind="ExternalInput").ap()
    wt_d = nc.dram_tensor("wt", [128, WTCOLS], F16, kind="ExternalInput").ap()
    wo_d = nc.dram_tensor("wo", [128, NDC * 1024], F16, kind="ExternalInput").ap()
    bo_d = nc.dram_tensor("bo", [128, 1024], FP32, kind="ExternalInput").ap()
    m_d = nc.dram_tensor("msk", [128, NBLK], FP32, kind="ExternalInput").ap()
    id_d = nc.dram_tensor("ident", [128, 128], F16, kind="ExternalInput").ap()
    out_d = nc.dram_tensor("out", [NBLK, 128, 1024], FP32, kind="ExternalOutput").ap()

    with tile.TileContext(nc) as tc:
        with (
            tc.tile_pool(name="const", bufs=1) as cp,
            tc.tile_pool(name="xp", bufs=3) as xp,
            tc.tile_pool(name="ep", bufs=3) as ep,
            tc.tile_pool(name="t1p", bufs=2) as t1p,
            tc.tile_pool(name="ap_", bufs=3) as apool,
            tc.tile_pool(name="atp", bufs=2) as atp,
            tc.tile_pool(name="op", bufs=2) as op,
            tc.tile_pool(name="sp", bufs=2) as sp,
            tc.tile_pool(name="pp", bufs=5, space="PSUM") as pp,
            tc.tile_pool(name="tp", bufs=1, space="PSUM") as tp,
            tc.tile_pool(name="outp", bufs=2, space="PSUM") as outp,
        ):
            wt = cp.tile([128, WTCOLS], F16, tag="wt")
            wo = cp.tile([128, NDC * 1024], F16, tag="wo")
            bo = cp.tile([128, 1024], FP32, tag="bo")
            msk = cp.tile([128, NBLK], FP32, tag="msk")
            ident = cp.tile([128, 128], F16, tag="ident")

            def load_wt(k):
                lo, hi = WTBASE[k], WTBASE[k] + NDC * CHUNKS[k][1]
                nc.sync.dma_start(wt[:, lo:hi], wt_d[:, lo:hi])

            def load_xt(i):
                t = xp.tile([128, 1024], F16, tag="xt")
                nc.sync.dma_start(t[:], xt_d[i])
                return t

            # DMA issue order tuned so data arrives roughly when needed:
            # x for the warmup blocks + early weight chunks first.
            xts = {0: load_xt(0)}
            load_wt(0)
            load_wt(1)
            xts[1] = load_xt(1)
            load_wt(2)
            load_wt(3)
            nc.sync.dma_start(msk[:], m_d[:])
            nc.sync.dma_start(ident[:], id_d[:])
            load_wt(4)
            load_wt(5)
            xts[2] = load_xt(2)
            load_wt(6)
            nc.sync.dma_start(wo[:], wo_d[:])
            nc.sync.dma_start(bo[:], bo_d[:])

            def proj_chunk(xt, k):
                """Accumulate projection chunk k into a psum tile."""
                off, sz = CHUNKS[k]
                ps = pp.tile([128, sz], FP32, tag="pp")
                for c in range(NDC):
                    nc.tensor.matmul(
                        ps[:], xt[:, c * 128:(c + 1) * 128],
                        wt[:, WTBASE[k] + c * sz: WTBASE[k] + (c + 1) * sz],
                        start=(c == 0), stop=(c == NDC - 1))
                return ps

            def finish_block(i, xt, e):
                """Everything after the exp chunks: v, stats, gate, a."""
                ps5 = proj_chunk(xt, 5)
                ps6 = proj_chunk(xt, 6)

                eq = e[:, 0:NQ].rearrange("p (h j) -> p h j", j=HD + 1)
                ek = e[:, NQ:QK].rearrange("p (h j) -> p h j", j=HD)
                denq = sp.tile([128, H], FP32, tag="denq")
                denk = sp.tile([128, H], FP32, tag="denk")
                eql = sp.tile([128, H], FP32, tag="eql")
                g = sp.tile([128, H], FP32, tag="g")
                nc.vector.tensor_reduce(denq[:], eq, axis=AX, op=ADD)
                nc.vector.tensor_reduce(denk[:], ek, axis=AX, op=ADD)
                nc.vector.tensor_copy(eql[:], eq[:, :, HD])
                nc.vector.tensor_sub(g[:], denq[:], eql[:])        # denq-eqlast
                nc.vector.tensor_mul(denq[:], denq[:], denk[:])    # denq*denk
                nc.vector.reciprocal(denk[:], denq[:])             # 1/(dq*dk)
                # msk holds 2*attention_mask -> G = 2*mask*(dq-el)/(dq*dk)
                nc.vector.scalar_tensor_tensor(
                    g[:], g[:], msk[:, i:i + 1], denk[:],
                    op0=mybir.AluOpType.mult, op1=mybir.AluOpType.mult)

                t1 = t1p.tile([128, 1024], FP32, tag="t1")
                nc.vector.tensor_mul(t1[:, 0:512], e[:, QK - 1024:QK - 512], ps5[:])
                nc.vector.tensor_mul(t1[:, 512:1024], e[:, QK - 512:QK], ps6[:])
                a = apool.tile([128, 1024], F16, tag="a")
                nc.vector.tensor_mul(
                    a[:].rearrange("p (h j) -> p h j", j=HD),
                    t1[:].rearrange("p (h j) -> p h j", j=HD),
                    g[:].to_broadcast((128, H, HD)))
                return a

            def tail(st, last=False):
                """Transpose a -> aT, final matmul + bias, DMA out."""
                i, a = st
                at = atp.tile([128, 1024], F16, tag="at")
                tps = tp.tile([128, 1024], F16, tag="tp")
                for c in range(NDC):
                    nc.tensor.matmul(tps[:, c * 128:(c + 1) * 128],
                                     a[:, c * 128:(c + 1) * 128], ident[:],
                                     is_transpose=True, skip_group_check=True)
                cp_eng = nc.vector.tensor_copy if last else nc.scalar.copy
                cp_eng(at[:, 0:512], tps[:, 0:512])
                cp_eng(at[:, 512:1024], tps[:, 512:1024])
                outsb = op.tile([128, 1024], FP32, tag="outsb")
                for half in range(2):
                    ops = outp.tile([128, 512], FP32, tag="outp")
                    for c in range(NDC):
                        nc.tensor.matmul(
                            ops[:], at[:, c * 128:(c + 1) * 128],
                            wo[:, c * 1024 + half * 512: c * 1024 + half * 512 + 512],
                            start=(c == 0), stop=(c == NDC - 1))
                    nc.vector.tensor_add(outsb[:, half * 512:(half + 1) * 512],
                                         ops[:], bo[:, half * 512:(half + 1) * 512])
                nc.sync.dma_start(out_d[i], outsb[:])

            # ---- warmup: blocks 0..WARM-1 chunk-major, tracking the
            # weight-chunk DMA arrival order ----
            es = {i: ep.tile([128, QK], F16, tag="e", name="e") for i in range(WARM)}
            for k in range(5):
                for i in range(WARM):
                    ps = proj_chunk(xts[i], k)
                    off, sz = CHUNKS[k]
                    nc.scalar.activation(es[i][:, off:off + sz], ps[:], EXP)
            pending = []
            for i in range(WARM):
                pending.append((i, finish_block(i, xts[i], es[i])))

            # ---- steady state ----
            for i in range(WARM, NBLK):
                xt = xts.get(i) or load_xt(i)
                e = ep.tile([128, QK], F16, tag="e")
                for k in range(4):
                    off, sz = CHUNKS[k]
                    ps = proj_chunk(xt, k)
                    nc.scalar.activation(e[:, off:off + sz], ps[:], EXP)
                ps4, ps5 = proj_pair45(xt)
                nc.scalar.activation(e[:, 2048:2064], ps4[:], EXP)
                pending.append((i, finish_block(i, xt, e, ps5)))
                tail(pending.pop(0))
            while pending:
                tail(pending.pop(0), last=True)
    nc.compile()
    return nc


def _host_prep(x, attention_mask, Wq, Wk, Wv, Wo, bo):
    x_flat = np.ascontiguousarray(np.asarray(x, dtype=np.float32)).reshape(ROWS, DIM)
    Wcat_T = np.ascontiguousarray(
        np.concatenate([np.asarray(Wq, np.float32), np.asarray(Wk, np.float32),
                        np.asarray(Wv, np.float32)], axis=0).T)  # [1024, 3088]
    cols = []
    for off, sz in CHUNKS:
        for c in range(NDC):
            cols.append(Wcat_T[c * 128:(c + 1) * 128, off:off + sz])
    wt_host = np.ascontiguousarray(np.concatenate(cols, axis=1)).astype(np.float16)

    wo_host = np.ascontiguousarray(
        np.asarray(Wo, np.float32).T.reshape(NDC, 128, 1024)
        .transpose(1, 0, 2).reshape(128, NDC * 1024)).astype(np.float16)
    bo_host = np.ascontiguousarray(
        np.broadcast_to(np.asarray(bo, np.float32), (128, 1024)))
    id_host = np.eye(128, dtype=np.float16)
    m_flat = (2.0 * np.asarray(attention_mask, np.float32)).reshape(ROWS)

    in_maps = []
    for i in range(NCORES):
        sl = slice(i * CROWS, (i + 1) * CROWS)
        xt = np.ascontiguousarray(
            x_flat[sl].reshape(NBLK, 128, NDC, 128).transpose(0, 3, 2, 1)
        ).reshape(NBLK, 128, 1024).astype(np.float16)
        mc = np.ascontiguousarray(m_flat[sl].reshape(NBLK, 128).T)
        in_maps.append({"xt": xt, "wt": wt_host, "wo": wo_host,
                        "bo": bo_host, "msk": mc, "ident": id_host})
    return in_maps


def run(inputs, trace=False):
    """Run the kernel; returns (output, exec_time_ns or None)."""
    if "nc" not in _CACHE:
        _CACHE["nc"] = _build()
    nc = _CACHE["nc"]
    in_maps = _host_prep(
        inputs["x"], inputs["attention_mask"], inputs["Wq"], inputs["Wk"],
        inputs["Wv"], inputs["Wo"], inputs["bo"])
    res = run_bass_kernel_spmd(nc, in_maps, list(range(NCORES)), trace=trace)
    out = np.concatenate(
        [res.results[i]["out"].reshape(CROWS, DIM) for i in range(NCORES)],
        axis=0).reshape(B, L, DIM)
    return out, res.exec_time_ns


def kernel(**inputs) -> np.ndarray:
    assert inputs["x"].shape == (B, L, DIM)
    out, _ = run(inputs, trace=False)
    return out


# revision 21
# speedup vs baseline: 1.0499x; 1.0004x over previous
"""Trainium2 Bass kernel for nn_Absolute_attention (sparse_attention).

Reference math (b=4, l=4096, dim=1024, h=16, hd=64):
    q = softmax((x @ Wq.T).reshape(b,l,h,hd+1), -1)
    time encoding: qk_weight = (1-q[...,-1]) * sum_d(time^2)  where
        sum_d(time[l,h,:]^2) = inv_hd * sum_j((c+s)^2 + (c-s)^2) = 2 exactly,
        so qk_weight = 2*(1-q_last)  (time/cos/sin cancel analytically).
    k = softmax((x @ Wk.T).reshape(b,l,h,hd), -1) * mask
    v = x @ Wv.T
    out = ((qk_weight[...,None]*k).reshape(b,l,h*hd) * v) @ Wo.T + bo

Everything is pointwise per (b,l) row -> pure data-parallel row sharding:
16384 rows over 8 cores = 2048 rows/core, 16 blocks of 128 rows.

Per 128-row block (layout: rows on partitions):
    z = x_blk @ Wcat.T  (Wcat = [Wq;Wk;Wv], 3088 cols) via PE fp16 matmuls,
        contraction over dim in 8 chunks of 128 (stationary = x.T chunks).
    e = exp(z[:, :2064])  (q+k logits; softmax without max-subtraction --
        logits are O(+-4), exp is safe in fp32)
    denq = segmented sum e_q (16 groups of 65); denk = seg sum e_k (16x64)
    G = 2*mask*(denq - eq_last) / (denq*denk)
    a = e_k * v * G[head-broadcast]   (fp16)
    aT = PE transpose of a (8x 128x128, into one fp16 PSUM bank)
    out = aT.T @ Wo.T + bo  via PE fp16 matmuls, then DMA out.

All matmul operands are fp16 (11-bit mantissa, ~3e-4 matmul err; 2-byte
weight loads keep the PE stream-bound and halve the weight DMA).
The first two blocks are processed chunk-major so compute overlaps the
8.4MB weight DMA stream instead of stalling behind it; later blocks run a
software pipeline (transposes of block i-2 -> projection of block i ->
final matmul of block i-2) that keeps the PE stream-bound.
"""
import numpy as np

import concourse.bacc as bacc
import concourse.mybir as mybir
import concourse.tile as tile
from concourse.bass_utils import run_bass_kernel_spmd

FP32 = mybir.dt.float32
F32R = mybir.dt.float32r
F16 = mybir.dt.float16
AX = mybir.AxisListType.X
ADD = mybir.AluOpType.add
EXP = mybir.ActivationFunctionType.Exp

B, L, DIM, H, HD = 4, 4096, 1024, 16, 64
ROWS = B * L                      # 16384
NCORES = 8
CROWS = ROWS // NCORES            # 2048
NBLK = CROWS // 128               # 16
NQ = H * (HD + 1)                 # 1040
NK = H * HD                       # 1024
QK = NQ + NK                      # 2064
TOT = QK + NK                     # 3088 (q | k | v)
NDC = DIM // 128                  # 8 contraction chunks

# N-chunks of the projection output; each fits one PSUM bank.
# First 5 cover the exp region [0, 2064), last 2 cover v. All exp chunks
# are >=256 wide so fp16 LDWEIGHTS (~100ns) hides fully under the streams.
CHUNKS = [(0, 416), (416, 416), (832, 416), (1248, 416), (1664, 400),
          (2064, 512), (2576, 512)]
WTBASE = []
_acc = 0
for _off, _sz in CHUNKS:
    WTBASE.append(_acc)
    _acc += NDC * _sz
WTCOLS = _acc                     # 24704

WARM = 2                          # blocks processed chunk-major at start

_CACHE = {}


def _build():
    nc = bacc.Bacc("TRN2", target_bir_lowering=False, debug=False)
    xt_d = nc.dram_tensor("xt", [NBLK, 128, 1024], F16, kind="ExternalInput").ap()
    wt_d = nc.dram_tensor("wt", [128, WTCOLS], F16, kind="ExternalInput").ap()
    wo_d = nc.dram_tensor("wo", [128, NDC * 1024], F16, kind="ExternalInput").ap()
    bo_d = nc.dram_tensor("bo", [128, 1024], FP32, kind="ExternalInput").ap()
    m_d = nc.dram_tensor("msk", [128, NBLK], FP32, kind="ExternalInput").ap()
    id_d = nc.dram_tensor("ident", [128, 128], F16, kind="ExternalInput").ap()
    out_d = nc.dram_tensor("out", [NBLK, 128, 1024], FP32, kind="ExternalOutput").ap()

    with tile.TileContext(nc) as tc:
        with (
            tc.tile_pool(name="const", bufs=1) as cp,
            tc.tile_pool(name="xp", bufs=3) as xp,
            tc.tile_pool(name="ep", bufs=3) as ep,
            tc.tile_pool(name="t1p", bufs=2) as t1p,
            tc.tile_pool(name="ap_", bufs=3) as apool,
            tc.tile_pool(name="atp", bufs=2) as atp,
            tc.tile_pool(name="op", bufs=2) as op,
            tc.tile_pool(name="sp", bufs=2) as sp,
            tc.tile_pool(name="pp", bufs=5, space="PSUM") as pp,
            tc.tile_pool(name="tp", bufs=1, space="PSUM") as tp,
            tc.tile_pool(name="outp", bufs=2, space="PSUM") as outp,
        ):
            wt = cp.tile([128, WTCOLS], F16, tag="wt")
            wo = cp.tile([128, NDC * 1024], F16, tag="wo")
            bo = cp.tile([128, 1024], FP32, tag="bo")
            msk = cp.tile([128, NBLK], FP32, tag="msk")
            ident = cp.tile([128, 128], F16, tag="ident")

            def load_wt(k):
                lo, hi = WTBASE[k], WTBASE[k] + NDC * CHUNKS[k][1]
                nc.sync.dma_start(wt[:, lo:hi], wt_d[:, lo:hi])

            def load_xt(i):
                t = xp.tile([128, 1024], F16, tag="xt")
                nc.sync.dma_start(t[:], xt_d[i])
                return t

            # DMA issue order tuned so data arrives roughly when needed:
            # x for the warmup blocks + early weight chunks first.
            xts = {0: load_xt(0)}
            load_wt(0)
            load_wt(1)
            xts[1] = load_xt(1)
            load_wt(2)
            load_wt(3)
            nc.sync.dma_start(msk[:], m_d[:])
            nc.sync.dma_start(ident[:], id_d[:])
            load_wt(4)
            load_wt(5)
            xts[2] = load_xt(2)
            load_wt(6)
            nc.sync.dma_start(wo[:], wo_d[:])
            nc.sync.dma_start(bo[:], bo_d[:])

            def proj_chunk(xt, k):
                """Accumulate projection chunk k into a psum tile."""
                off, sz = CHUNKS[k]
                ps = pp.tile([128, sz], FP32, tag="pp")
                for c in range(NDC):
                    nc.tensor.matmul(
                        ps[:], xt[:, c * 128:(c + 1) * 128],
                        wt[:, WTBASE[k] + c * sz: WTBASE[k] + (c + 1) * sz],
                        start=(c == 0), stop=(c == NDC - 1))
                return ps

            def finish_block(i, xt, e):
                """Everything after the exp chunks: v, stats, gate, a."""
                ps5 = proj_chunk(xt, 5)
                ps6 = proj_chunk(xt, 6)

                eq = e[:, 0:NQ].rearrange("p (h j) -> p h j", j=HD + 1)
                ek = e[:, NQ:QK].rearrange("p (h j) -> p h j", j=HD)
                denq = sp.tile([128, H], FP32, tag="denq")
                denk = sp.tile([128, H], FP32, tag="denk")
                eql = sp.tile([128, H], FP32, tag="eql")
                g = sp.tile([128, H], FP32, tag="g")
                nc.vector.tensor_reduce(denq[:], eq, axis=AX, op=ADD)
                nc.vector.tensor_reduce(denk[:], ek, axis=AX, op=ADD)
                nc.vector.tensor_copy(eql[:], eq[:, :, HD])
                nc.vector.tensor_sub(g[:], denq[:], eql[:])        # denq-eqlast
                nc.vector.tensor_mul(denq[:], denq[:], denk[:])    # denq*denk
                nc.vector.reciprocal(denk[:], denq[:])             # 1/(dq*dk)
                # msk holds 2*attention_mask -> G = 2*mask*(dq-el)/(dq*dk)
                nc.vector.scalar_tensor_tensor(
                    g[:], g[:], msk[:, i:i + 1], denk[:],
                    op0=mybir.AluOpType.mult, op1=mybir.AluOpType.mult)

                t1 = t1p.tile([128, 1024], FP32, tag="t1")
                nc.vector.tensor_mul(t1[:, 0:512], e[:, QK - 1024:QK - 512], ps5[:])
                nc.vector.tensor_mul(t1[:, 512:1024], e[:, QK - 512:QK], ps6[:])
                a = apool.tile([128, 1024], F16, tag="a")
                nc.vector.tensor_mul(
                    a[:].rearrange("p (h j) -> p h j", j=HD),
                    t1[:].rearrange("p (h j) -> p h j", j=HD),
                    g[:].to_broadcast((128, H, HD)))
                return a

            def tail(st, last=False):
                """Transpose a -> aT, final matmul + bias, DMA out."""
                i, a = st
                at = atp.tile([128, 1024], F16, tag="at")
                tps = tp.tile([128, 1024], F16, tag="tp")
                for c in range(NDC):
                    nc.tensor.matmul(tps[:, c * 128:(c + 1) * 128],
                                     a[:, c * 128:(c + 1) * 128], ident[:],
                                     is_transpose=True, skip_group_check=True)
                cp_eng = nc.vector.tensor_copy if last else nc.scalar.copy
                cp_eng(at[:, 0:512], tps[:, 0:512])
                cp_eng(at[:, 512:1024], tps[:, 512:1024])
                outsb = op.tile([128, 1024], FP32, tag="outsb")
                for half in range(2):
                    ops = outp.tile([128, 512], FP32, tag="outp")
                    for c in range(NDC):
                        nc.tensor.matmul(
                            ops[:], at[:, c * 128:(c + 1) * 128],
                            wo[:, c * 1024 + half * 512: c * 1024 + half * 512 + 512],
                            start=(c == 0), stop=(c == NDC - 1))
                    nc.vector.tensor_add(outsb[:, half * 512:(half + 1) * 512],
                                         ops[:], bo[:, half * 512:(half + 1) * 512])
                nc.sync.dma_start(out_d[i], outsb[:])

            # ---- warmup: blocks 0..WARM-1 chunk-major, tracking the
            # weight-chunk DMA arrival order ----
            es = {i: ep.tile([128, QK], F16, tag="e", name="e") for i in range(WARM)}
            for k in range(5):
                for i in range(WARM):
                    ps = proj_chunk(xts[i], k)
                    off, sz = CHUNKS[k]
                    nc.scalar.activation(es[i][:, off:off + sz], ps[:], EXP)
            pending = []
            for i in range(WARM):
                pending.append((i, finish_block(i, xts[i], es[i])))

            # ---- steady state ----
            for i in range(WARM, NBLK):
                xt = xts.get(i) or load_xt(i)
                e = ep.tile([128, QK], F16, tag="e")
                for k in range(4):
                    off, sz = CHUNKS[k]
                    ps = proj_chunk(xt, k)
                    nc.scalar.activation(e[:, off:off + sz], ps[:], EXP)
                ps4, ps5 = proj_pair45(xt)
                nc.scalar.activation(e[:, 2048:2064], ps4[:], EXP)
                pending.append((i, finish_block(i, xt, e, ps5)))
                tail(pending.pop(0))
            while pending:
                tail(pending.pop(0), last=True)
    nc.compile()
    return nc


def _host_prep(x, attention_mask, Wq, Wk, Wv, Wo, bo):
    x_flat = np.ascontiguousarray(np.asarray(x, dtype=np.float32)).reshape(ROWS, DIM)
    Wcat_T = np.ascontiguousarray(
        np.concatenate([np.asarray(Wq, np.float32), np.asarray(Wk, np.float32),
                        np.asarray(Wv, np.float32)], axis=0).T)  # [1024, 3088]
    cols = []
    for off, sz in CHUNKS:
        for c in range(NDC):
            cols.append(Wcat_T[c * 128:(c + 1) * 128, off:off + sz])
    wt_host = np.ascontiguousarray(np.concatenate(cols, axis=1)).astype(np.float16)

    wo_host = np.ascontiguousarray(
        np.asarray(Wo, np.float32).T.reshape(NDC, 128, 1024)
        .transpose(1, 0, 2).reshape(128, NDC * 1024)).astype(np.float16)
    bo_host = np.ascontiguousarray(
        np.broadcast_to(np.asarray(bo, np.float32), (128, 1024)))
    id_host = np.eye(128, dtype=np.float16)
    m_flat = (2.0 * np.asarray(attention_mask, np.float32)).reshape(ROWS)

    in_maps = []
    for i in range(NCORES):
        sl = slice(i * CROWS, (i + 1) * CROWS)
        xt = np.ascontiguousarray(
            x_flat[sl].reshape(NBLK, 128, NDC, 128).transpose(0, 3, 2, 1)
        ).reshape(NBLK, 128, 1024).astype(np.float16)
        mc = np.ascontiguousarray(m_flat[sl].reshape(NBLK, 128).T)
        in_maps.append({"xt": xt, "wt": wt_host, "wo": wo_host,
                        "bo": bo_host, "msk": mc, "ident": id_host})
    return in_maps


def run(inputs, trace=False):
    """Run the kernel; returns (output, exec_time_ns or None)."""
    if "nc" not in _CACHE:
        _CACHE["nc"] = _build()
    nc = _CACHE["nc"]
    in_maps = _host_prep(
        inputs["x"], inputs["attention_mask"], inputs["Wq"], inputs["Wk"],
        inputs["Wv"], inputs["Wo"], inputs["bo"])
    res = None
    for attempt in range(3):
        try:
            res = run_bass_kernel_spmd(nc, in_maps, list(range(NCORES)),
                                       trace=trace)
            break
        except Exception:
            # rare transient NRT_EXEC_UNIT_UNRECOVERABLE; device recovers
            if attempt == 2:
                raise
            import time as _time
            _time.sleep(10)
    out = np.concatenate(
        [res.results[i]["out"].reshape(CROWS, DIM) for i in range(NCORES)],
        axis=0).reshape(B, L, DIM)
    return out, res.exec_time_ns


def kernel(**inputs) -> np.ndarray:
    assert inputs["x"].shape == (B, L, DIM)
    out, _ = run(inputs, trace=False)
    return out


# revision 23
# speedup vs baseline: 1.0567x; 1.0064x over previous
"""Trainium2 Bass kernel for nn_Absolute_attention (sparse_attention).

Reference math (b=4, l=4096, dim=1024, h=16, hd=64):
    q = softmax((x @ Wq.T).reshape(b,l,h,hd+1), -1)
    time encoding: qk_weight = (1-q[...,-1]) * sum_d(time^2)  where
        sum_d(time[l,h,:]^2) = inv_hd * sum_j((c+s)^2 + (c-s)^2) = 2 exactly,
        so qk_weight = 2*(1-q_last)  (time/cos/sin cancel analytically).
    k = softmax((x @ Wk.T).reshape(b,l,h,hd), -1) * mask
    v = x @ Wv.T
    out = ((qk_weight[...,None]*k).reshape(b,l,h*hd) * v) @ Wo.T + bo

Everything is pointwise per (b,l) row -> pure data-parallel row sharding:
16384 rows over 8 cores = 2048 rows/core, 16 blocks of 128 rows.

Per 128-row block (layout: rows on partitions):
    z = x_blk @ Wcat.T  (Wcat = [Wq;Wk;Wv], 3088 cols) via PE fp16 matmuls,
        contraction over dim in 8 chunks of 128 (stationary = x.T chunks).
    e = exp(z[:, :2064])  (q+k logits; softmax without max-subtraction --
        logits are O(+-4), exp is safe in fp32)
    denq = segmented sum e_q (16 groups of 65); denk = seg sum e_k (16x64)
    G = 2*mask*(denq - eq_last) / (denq*denk)
    a = e_k * v * G[head-broadcast]   (fp16)
    aT = PE transpose of a (8x 128x128, into one fp16 PSUM bank)
    out = aT.T @ Wo.T + bo  via PE fp16 matmuls, then DMA out.

All matmul operands are fp16 (11-bit mantissa, ~3e-4 matmul err; 2-byte
weight loads keep the PE stream-bound and halve the weight DMA).
The first two blocks are processed chunk-major so compute overlaps the
8.4MB weight DMA stream instead of stalling behind it; later blocks run a
software pipeline (transposes of block i-2 -> projection of block i ->
final matmul of block i-2) that keeps the PE stream-bound.
"""
import numpy as np

import concourse.bacc as bacc
import concourse.mybir as mybir
import concourse.tile as tile
from concourse.bass_utils import run_bass_kernel_spmd

FP32 = mybir.dt.float32
F32R = mybir.dt.float32r
F16 = mybir.dt.float16
AX = mybir.AxisListType.X
ADD = mybir.AluOpType.add
EXP = mybir.ActivationFunctionType.Exp

B, L, DIM, H, HD = 4, 4096, 1024, 16, 64
ROWS = B * L                      # 16384
NCORES = 8
CROWS = ROWS // NCORES            # 2048
NBLK = CROWS // 128               # 16
NQ = H * (HD + 1)                 # 1040
NK = H * HD                       # 1024
QK = NQ + NK                      # 2064
TOT = QK + NK                     # 3088 (q | k | v)
NDC = DIM // 128                  # 8 contraction chunks

# N-chunks of the projection output; each fits one PSUM bank.
# First 5 cover the exp region [0, 2064), last 2 cover v. All exp chunks
# are >=256 wide so fp16 LDWEIGHTS (~100ns) hides fully under the streams.
CHUNKS = [(0, 416), (416, 416), (832, 416), (1248, 416), (1664, 400),
          (2064, 512), (2576, 512)]
WTBASE = []
_acc = 0
for _off, _sz in CHUNKS:
    WTBASE.append(_acc)
    _acc += NDC * _sz
WTCOLS = _acc                     # 24704

WARM = 2                          # blocks processed chunk-major at start

_CACHE = {}


def _build():
    nc = bacc.Bacc("TRN2", target_bir_lowering=False, debug=False)
    xt_d = nc.dram_tensor("xt", [NBLK, 128, 1024], F16, kind="ExternalInput").ap()
    wt_d = nc.dram_tensor("wt", [128, WTCOLS], F16, kind="ExternalInput").ap()
    wo_d = nc.dram_tensor("wo", [128, NDC * 1024], F16, kind="ExternalInput").ap()
    bo_d = nc.dram_tensor("bo", [128, 1024], FP32, kind="ExternalInput").ap()
    m_d = nc.dram_tensor("msk", [128, NBLK], FP32, kind="ExternalInput").ap()
    id_d = nc.dram_tensor("ident", [128, 128], F16, kind="ExternalInput").ap()
    out_d = nc.dram_tensor("out", [NBLK, 128, 1024], FP32, kind="ExternalOutput").ap()

    with tile.TileContext(nc) as tc:
        with (
            tc.tile_pool(name="const", bufs=1) as cp,
            tc.tile_pool(name="xp", bufs=3) as xp,
            tc.tile_pool(name="ep", bufs=3) as ep,
            tc.tile_pool(name="t1p", bufs=2) as t1p,
            tc.tile_pool(name="ap_", bufs=3) as apool,
            tc.tile_pool(name="atp", bufs=2) as atp,
            tc.tile_pool(name="op", bufs=2) as op,
            tc.tile_pool(name="sp", bufs=2) as sp,
            tc.tile_pool(name="pp", bufs=5, space="PSUM") as pp,
            tc.tile_pool(name="tp", bufs=1, space="PSUM") as tp,
            tc.tile_pool(name="outp", bufs=2, space="PSUM") as outp,
        ):
            wt = cp.tile([128, WTCOLS], F16, tag="wt")
            wo = cp.tile([128, NDC * 1024], F16, tag="wo")
            bo = cp.tile([128, 1024], FP32, tag="bo")
            msk = cp.tile([128, NBLK], FP32, tag="msk")
            ident = cp.tile([128, 128], F16, tag="ident")

            def load_wt(k):
                lo, hi = WTBASE[k], WTBASE[k] + NDC * CHUNKS[k][1]
                nc.sync.dma_start(wt[:, lo:hi], wt_d[:, lo:hi])

            def load_xt(i):
                t = xp.tile([128, 1024], F16, tag="xt")
                nc.sync.dma_start(t[:], xt_d[i])
                return t

            # DMA issue order tuned so data arrives roughly when needed:
            # x for the warmup blocks + early weight chunks first.
            xts = {0: load_xt(0)}
            load_wt(0)
            load_wt(1)
            xts[1] = load_xt(1)
            load_wt(2)
            load_wt(3)
            nc.sync.dma_start(msk[:], m_d[:])
            nc.sync.dma_start(ident[:], id_d[:])
            load_wt(4)
            load_wt(5)
            xts[2] = load_xt(2)
            load_wt(6)
            nc.sync.dma_start(wo[:], wo_d[:])
            nc.sync.dma_start(bo[:], bo_d[:])

            def proj_chunk(xt, k):
                """Accumulate projection chunk k into a psum tile."""
                off, sz = CHUNKS[k]
                ps = pp.tile([128, sz], FP32, tag="pp")
                for c in range(NDC):
                    nc.tensor.matmul(
                        ps[:], xt[:, c * 128:(c + 1) * 128],
                        wt[:, WTBASE[k] + c * sz: WTBASE[k] + (c + 1) * sz],
                        start=(c == 0), stop=(c == NDC - 1))
                return ps

            def finish_block(i, xt, e):
                """Everything after the exp chunks: v, stats, gate, a."""
                ps5 = proj_chunk(xt, 5)
                ps6 = proj_chunk(xt, 6)

                eq = e[:, 0:NQ].rearrange("p (h j) -> p h j", j=HD + 1)
                ek = e[:, NQ:QK].rearrange("p (h j) -> p h j", j=HD)
                denq = sp.tile([128, H], FP32, tag="denq")
                denk = sp.tile([128, H], FP32, tag="denk")
                eql = sp.tile([128, H], FP32, tag="eql")
                g = sp.tile([128, H], FP32, tag="g")
                nc.vector.tensor_reduce(denq[:], eq, axis=AX, op=ADD)
                nc.vector.tensor_reduce(denk[:], ek, axis=AX, op=ADD)
                nc.vector.tensor_copy(eql[:], eq[:, :, HD])
                nc.vector.tensor_sub(g[:], denq[:], eql[:])        # denq-eqlast
                nc.vector.tensor_mul(denq[:], denq[:], denk[:])    # denq*denk
                nc.vector.reciprocal(denk[:], denq[:])             # 1/(dq*dk)
                # msk holds 2*attention_mask -> G = 2*mask*(dq-el)/(dq*dk)
                nc.vector.scalar_tensor_tensor(
                    g[:], g[:], msk[:, i:i + 1], denk[:],
                    op0=mybir.AluOpType.mult, op1=mybir.AluOpType.mult)

                t1 = t1p.tile([128, 1024], FP32, tag="t1")
                nc.vector.tensor_mul(t1[:, 0:512], e[:, QK - 1024:QK - 512], ps5[:])
                nc.vector.tensor_mul(t1[:, 512:1024], e[:, QK - 512:QK], ps6[:])
                a = apool.tile([128, 1024], F16, tag="a")
                nc.vector.tensor_mul(
                    a[:].rearrange("p (h j) -> p h j", j=HD),
                    t1[:].rearrange("p (h j) -> p h j", j=HD),
                    g[:].to_broadcast((128, H, HD)))
                return a

            def tail(st, last=False):
                """Transpose a -> aT, final matmul + bias, DMA out."""
                i, a = st
                at = atp.tile([128, 1024], F16, tag="at")
                tps = tp.tile([128, 1024], F16, tag="tp")
                for c in range(NDC):
                    nc.tensor.matmul(tps[:, c * 128:(c + 1) * 128],
                                     a[:, c * 128:(c + 1) * 128], ident[:],
                                     is_transpose=True, skip_group_check=True)
                cp_eng = nc.vector.tensor_copy if last else nc.scalar.copy
                cp_eng(at[:, 0:512], tps[:, 0:512])
                cp_eng(at[:, 512:1024], tps[:, 512:1024])
                outsb = op.tile([128, 1024], FP32, tag="outsb")
                for half in range(2):
                    ops = outp.tile([128, 512], FP32, tag="outp")
                    for c in range(NDC):
                        nc.tensor.matmul(
                            ops[:], at[:, c * 128:(c + 1) * 128],
                            wo[:, c * 1024 + half * 512: c * 1024 + half * 512 + 512],
                            start=(c == 0), stop=(c == NDC - 1))
                    nc.vector.tensor_add(outsb[:, half * 512:(half + 1) * 512],
                                         ops[:], bo[:, half * 512:(half + 1) * 512])
                nc.sync.dma_start(out_d[i], outsb[:])

            # ---- warmup: blocks 0..WARM-1 chunk-major, tracking the
            # weight-chunk DMA arrival order ----
            es = {i: ep.tile([128, QK], F16, tag="e", name="e") for i in range(WARM)}
            for k in range(5):
                for i in range(WARM):
                    ps = proj_chunk(xts[i], k)
                    off, sz = CHUNKS[k]
                    nc.scalar.activation(es[i][:, off:off + sz], ps[:], EXP)
            pending = []
            for i in range(WARM):
                pending.append((i, finish_block(i, xts[i], es[i])))

            # ---- steady state ----
            for i in range(WARM, NBLK):
                xt = xts.get(i) or load_xt(i)
                e = ep.tile([128, QK], F16, tag="e")
                for k in range(4):
                    off, sz = CHUNKS[k]
                    ps = proj_chunk(xt, k)
                    nc.scalar.activation(e[:, off:off + sz], ps[:], EXP)
                ps4, ps5 = proj_pair45(xt)
                nc.scalar.activation(e[:, 2048:2064], ps4[:], EXP)
                pending.append((i, finish_block(i, xt, e, ps5)))
                tail(pending.pop(0))
            while pending:
                tail(pending.pop(0), last=True)
    nc.compile()
    return nc


def _host_prep(x, attention_mask, Wq, Wk, Wv, Wo, bo):
    x_flat = np.ascontiguousarray(np.asarray(x, dtype=np.float32)).reshape(ROWS, DIM)
    Wcat_T = np.ascontiguousarray(
        np.concatenate([np.asarray(Wq, np.float32), np.asarray(Wk, np.float32),
                        np.asarray(Wv, np.float32)], axis=0).T)  # [1024, 3088]
    cols = []
    for off, sz in CHUNKS:
        for c in range(NDC):
            cols.append(Wcat_T[c * 128:(c + 1) * 128, off:off + sz])
    wt_host = np.ascontiguousarray(np.concatenate(cols, axis=1)).astype(np.float16)

    wo_host = np.ascontiguousarray(
        np.asarray(Wo, np.float32).T.reshape(NDC, 128, 1024)
        .transpose(1, 0, 2).reshape(128, NDC * 1024)).astype(np.float16)
    bo_host = np.ascontiguousarray(
        np.broadcast_to(np.asarray(bo, np.float32), (128, 1024)))
    id_host = np.eye(128, dtype=np.float16)
    m_flat = (2.0 * np.asarray(attention_mask, np.float32)).reshape(ROWS)

    in_maps = []
    for i in range(NCORES):
        sl = slice(i * CROWS, (i + 1) * CROWS)
        xt = np.ascontiguousarray(
            x_flat[sl].reshape(NBLK, 128, NDC, 128).transpose(0, 3, 2, 1)
        ).reshape(NBLK, 128, 1024).astype(np.float16)
        mc = np.ascontiguousarray(m_flat[sl].reshape(NBLK, 128).T)
        in_maps.append({"xt": xt, "wt": wt_host, "wo": wo_host,
                        "bo": bo_host, "msk": mc, "ident": id_host})
    return in_maps


def run(inputs, trace=False):
    """Run the kernel; returns (output, exec_time_ns or None)."""
    if "nc" not in _CACHE:
        _CACHE["nc"] = _build()
    nc = _CACHE["nc"]
    in_maps = _host_prep(
        inputs["x"], inputs["attention_mask"], inputs["Wq"], inputs["Wk"],
        inputs["Wv"], inputs["Wo"], inputs["bo"])
    res = None
    for attempt in range(3):
        try:
            res = run_bass_kernel_spmd(nc, in_maps, list(range(NCORES)),
                                       trace=trace)
            break
        except Exception:
            # rare transient NRT_EXEC_UNIT_UNRECOVERABLE; device recovers
            if attempt == 2:
                raise
            import time as _time
            _time.sleep(10)
    out = np.concatenate(
        [res.results[i]["out"].reshape(CROWS, DIM) for i in range(NCORES)],
        axis=0).reshape(B, L, DIM)
    return out, res.exec_time_ns


def kernel(**inputs) -> np.ndarray:
    assert inputs["x"].shape == (B, L, DIM)
    out, _ = run(inputs, trace=False)
    return out
